# revision 1
# baseline (speedup 1.0000x reference)
"""SimCLR contrastive-loss kernel for 8 Trainium2 NeuronCores.

Full inputs in, full outputs out.  Internally: shard proj_1/proj_2 rows
across the 8 cores; each core normalizes+transposes its proj_2 shard on
the PE (scale folded into a diag matmul), AllGathers the normalized
z2^T, computes its 1024x8192 row-block of the similarity matrix with
float32r matmuls, does a streaming logsumexp (per-2048-group max on DVE,
exp+row-sum on ACT with per-partition bias, exact group-shift fixup),
and writes 1024 per-row losses + 1024 positives.  Host sums the partials.
"""

import os
import numpy as np

DEBUG_NO_CC = bool(os.environ.get("K_NO_CC"))

B = 8192          # batch
D = 256           # feature dim
NCORES = 8
R = B // NCORES   # rows per core = 1024
P = 128           # partitions
MT = R // P       # M-tiles per core = 8
GROUP = 1024      # columns per logsumexp group
NG = B // GROUP   # groups per row = 8
NS = 512          # matmul moving free dim
TEMP_INV = 1000.0

_CACHE = {}


def _build_nc():
    import concourse.bacc as bacc
    import concourse.mybir as mybir
    from concourse import tile, masks

    f32 = mybir.dt.float32
    f32r = mybir.dt.float32r
    AOT = mybir.AluOpType
    ACT = mybir.ActivationFunctionType

    nc = bacc.Bacc("TRN2", target_bir_lowering=False, debug=False,
                   num_devices=NCORES)

    p1 = nc.dram_tensor("p1", [R, D], f32, kind="ExternalInput")
    p2s = nc.dram_tensor("p2s", [R, D], f32, kind="ExternalInput")
    res = nc.dram_tensor("res", [P, 2 * MT], f32, kind="ExternalOutput")
    ag_in = nc.dram_tensor("ag_in", [D, R], f32r, kind="Internal")
    ag_out = nc.dram_tensor("ag_out", [NCORES * D, R], f32r, kind="Internal",
                            addr_space="Shared")
    rg = [list(range(NCORES))]

    with tile.TileContext(nc) as tc:
        with (
            tc.tile_pool(name="big", bufs=1) as big,
            tc.tile_pool(name="scr", bufs=2) as scr,
            tc.tile_pool(name="dscr", bufs=4) as dscr,
        ):
            # persistent SBUF tensors
            z2T0 = big.tile([P, B], f32r, tag="z2T0")  # z2^T dims 0..127
            z2T1 = big.tile([P, B], f32r, tag="z2T1")  # z2^T dims 128..255
            xT0 = big.tile([P, R], f32r, tag="xT0")    # x^T dims 0..127
            xT1 = big.tile([P, R], f32r, tag="xT1")
            xs = big.tile([P, MT * D], f32, tag="xs")  # p1 natural tiles
            ys = big.tile([P, MT * D], f32, tag="ys")  # p2 shard natural tiles
            zsh0 = big.tile([P, R], f32r, tag="zsh0")  # staged z2T shard
            zsh1 = big.tile([P, R], f32r, tag="zsh1")
            ident = big.tile([P, P], f32, tag="ident")
            n2x = big.tile([P, MT], f32, tag="n2x")
            n2y = big.tile([P, MT], f32, tag="n2y")
            rix = big.tile([P, MT], f32, tag="rix")
            riy = big.tile([P, MT], f32, tag="riy")
            tln = big.tile([P, MT], f32, tag="tln")
            rr = big.tile([P, MT], f32, tag="rr")
            praw = big.tile([P, MT], f32, tag="praw")
            posv = big.tile([P, MT], f32, tag="posv")
            gmax = big.tile([P, MT * NG], f32, tag="gmax")
            negb = big.tile([P, MT * NG], f32, tag="negb")
            ssum = big.tile([P, MT * NG], f32, tag="ssum")
            t4 = big.tile([P, MT * NG], f32, tag="t4")
            st4 = big.tile([P, MT * NG], f32, tag="st4")
            mrow = big.tile([P, MT], f32, tag="mrow")
            negm = big.tile([P, MT], f32, tag="negm")
            stot = big.tile([P, MT], f32, tag="stot")
            lnst = big.tile([P, MT], f32, tag="lnst")
            wdif = big.tile([P, MT], f32, tag="wdif")
            outt = big.tile([P, 2 * MT], f32, tag="outt")

            masks.make_identity(nc, ident[:])

            # ---------------- phase A: p2 shard -> normalized z2T shard
            with tc.tile_pool(name="ppsum", bufs=2, space="PSUM") as ppsum:
                for t in range(MT):
                    nc.sync.dma_start(ys[:, t * D:(t + 1) * D],
                                      p2s[t * P:(t + 1) * P, :])
                for t in range(MT):
                    sq = scr.tile([P, D], f32, tag="sq")
                    nc.scalar.activation(sq[:], ys[:, t * D:(t + 1) * D],
                                         ACT.Square,
                                         accum_out=n2y[:, t:t + 1])
                # 1/sqrt(s) = exp(-0.5*ln(s)) (exp+ln share one ACT table set)
                nc.scalar.activation(tln[:], n2y[:], ACT.Ln)
                nc.scalar.activation(riy[:], tln[:], ACT.Exp, scale=-0.5)
                pt0 = ppsum.tile([P, R], f32, tag="pt0")
                pt1 = ppsum.tile([P, R], f32, tag="pt1")
                for t in range(MT):
                    dg = dscr.tile([P, P], f32, tag="dg")
                    nc.gpsimd.tensor_scalar_mul(dg[:], ident[:],
                                                riy[:, t:t + 1])
                    nc.tensor.matmul(pt0[:, t * P:(t + 1) * P],
                                     ys[:, t * D:t * D + P], dg[:])
                    nc.tensor.matmul(pt1[:, t * P:(t + 1) * P],
                                     ys[:, t * D + P:(t + 1) * D], dg[:])
                nc.vector.tensor_copy(zsh0[:], pt0[:])
                nc.vector.tensor_copy(zsh1[:], pt1[:])

                # ---------------- phase B: p1 shard -> normalized x^T
                for m in range(MT):
                    nc.sync.dma_start(xs[:, m * D:(m + 1) * D],
                                      p1[m * P:(m + 1) * P, :])
                for m in range(MT):
                    sq = scr.tile([P, D], f32, tag="sq")
                    nc.scalar.activation(sq[:], xs[:, m * D:(m + 1) * D],
                                         ACT.Square,
                                         accum_out=n2x[:, m:m + 1])
                nc.scalar.activation(tln[:], n2x[:], ACT.Ln)
                nc.scalar.activation(rix[:], tln[:], ACT.Exp, scale=-0.5)
                pt0 = ppsum.tile([P, R], f32, tag="pt0")
                pt1 = ppsum.tile([P, R], f32, tag="pt1")
                for m in range(MT):
                    dg = dscr.tile([P, P], f32, tag="dg")
                    nc.gpsimd.tensor_scalar_mul(dg[:], ident[:],
                                                rix[:, m:m + 1])
                    nc.tensor.matmul(pt0[:, m * P:(m + 1) * P],
                                     xs[:, m * D:m * D + P], dg[:])
                    nc.tensor.matmul(pt1[:, m * P:(m + 1) * P],
                                     xs[:, m * D + P:(m + 1) * D], dg[:])
                nc.vector.tensor_copy(xT0[:], pt0[:])
                nc.vector.tensor_copy(xT1[:], pt1[:])

            # ship z2T shard to DRAM, AllGather, pull full z2T back
            nc.sync.dma_start(ag_in[0:P, :], zsh0[:])
            nc.sync.dma_start(ag_in[P:D, :], zsh1[:])
            if DEBUG_NO_CC:
                # debug: skip collective AND the Shared buffer; replicate the
                # local shard from ag_in (Local scratchpad)
                for c in range(NCORES):
                    nc.sync.dma_start(z2T0[:, c * R:(c + 1) * R],
                                      ag_in[0:P, :])
                    nc.sync.dma_start(z2T1[:, c * R:(c + 1) * R],
                                      ag_in[P:D, :])
            else:
                nc.gpsimd.collective_compute(
                    "AllGather", AOT.bypass, replica_groups=rg,
                    ins=[ag_in.ap()], outs=[ag_out.ap()])
                for c in range(NCORES):
                    nc.sync.dma_start(z2T0[:, c * R:(c + 1) * R],
                                      ag_out[c * D:c * D + P, :])
                    nc.sync.dma_start(z2T1[:, c * R:(c + 1) * R],
                                      ag_out[c * D + P:(c + 1) * D, :])

            # ---------------- positives (exact fp32, from raw shards)
            nc.vector.tensor_mul(rr[:], rix[:], riy[:])
            for m in range(MT):
                sq = scr.tile([P, D], f32, tag="sq")
                nc.vector.tensor_mul(sq[:], xs[:, m * D:(m + 1) * D],
                                     ys[:, m * D:(m + 1) * D])
                nc.vector.reduce_sum(out=praw[:, m:m + 1], in_=sq[:],
                                     axis=mybir.AxisListType.X)
            nc.vector.tensor_mul(posv[:], praw[:], rr[:])

            # ---------------- main loop: row-block logsumexp
            xTk = (xT0, xT1)
            zTk = (z2T0, z2T1)
            with (tc.tile_pool(name="mpsum", bufs=4, space="PSUM") as mpsum,
                  tc.tile_pool(name="escr", bufs=3) as escr):
                for m in range(MT):
                    for g in range(NG):
                        col = m * NG + g
                        pg = mpsum.tile([P, GROUP], f32, tag="pg")
                        for n in range(GROUP // NS):
                            for k in range(2):
                                nc.tensor.matmul(
                                    pg[:, n * NS:(n + 1) * NS],
                                    xTk[k][:, m * P:(m + 1) * P],
                                    zTk[k][:, g * GROUP + n * NS:
                                           g * GROUP + (n + 1) * NS],
                                    start=(k == 0), stop=(k == 1))
                        nc.vector.reduce_max(out=gmax[:, col:col + 1],
                                             in_=pg[:],
                                             axis=mybir.AxisListType.X)
                        nc.gpsimd.tensor_scalar_mul(negb[:, col:col + 1],
                                                    gmax[:, col:col + 1],
                                                    -TEMP_INV)
                        eo = escr.tile([P, GROUP], f32, tag="eo")
                        nc.scalar.activation(eo[:], pg[:], ACT.Exp,
                                             scale=TEMP_INV,
                                             bias=negb[:, col:col + 1],
                                             accum_out=ssum[:, col:col + 1])
                    # per-M-tile fixup: combine the NG group sums exactly
                    c0, c1 = m * NG, (m + 1) * NG
                    nc.vector.reduce_max(out=mrow[:, m:m + 1],
                                         in_=gmax[:, c0:c1],
                                         axis=mybir.AxisListType.X)
                    nc.gpsimd.tensor_scalar_mul(negm[:, m:m + 1],
                                                mrow[:, m:m + 1], -TEMP_INV)
                    nc.scalar.activation(t4[:, c0:c1], gmax[:, c0:c1],
                                         ACT.Exp, scale=TEMP_INV,
                                         bias=negm[:, m:m + 1])
                    nc.vector.tensor_mul(st4[:, c0:c1], t4[:, c0:c1],
                                         ssum[:, c0:c1])
                    nc.vector.reduce_sum(out=stot[:, m:m + 1],
                                         in_=st4[:, c0:c1],
                                         axis=mybir.AxisListType.X)
                    nc.scalar.activation(lnst[:, m:m + 1], stot[:, m:m + 1],
                                         ACT.Ln)

            # all_losses = ln(stot) + 1000*(mrow - pos);  outputs
            nc.vector.tensor_sub(wdif[:], mrow[:], posv[:])
            nc.vector.tensor_scalar_mul(wdif[:], wdif[:], TEMP_INV)
            nc.vector.tensor_add(outt[:, 0:MT], wdif[:], lnst[:])
            nc.vector.tensor_copy(outt[:, MT:2 * MT], posv[:])
            nc.sync.dma_start(res[:, :], outt[:])

    nc.compile()
    return nc


def _get_nc():
    if "nc" not in _CACHE:
        _CACHE["nc"] = _build_nc()
    return _CACHE["nc"]


def run_cores(proj_1, proj_2, **spmd_kwargs):
    """Run the SPMD kernel; returns (per-core results list, BassKernelResults)."""
    from concourse.bass_utils import run_bass_kernel_spmd

    p1 = np.ascontiguousarray(np.asarray(proj_1, dtype=np.float32))
    p2 = np.ascontiguousarray(np.asarray(proj_2, dtype=np.float32))
    assert p1.shape == (B, D) and p2.shape == (B, D)
    in_maps = [
        {"p1": p1[c * R:(c + 1) * R], "p2s": p2[c * R:(c + 1) * R]}
        for c in range(NCORES)
    ]
    nc = _get_nc()
    br = run_bass_kernel_spmd(nc, in_maps, core_ids=list(range(NCORES)),
                              **spmd_kwargs)
    return br


def kernel(proj_1, proj_2):
    br = run_cores(proj_1, proj_2)
    loss_sum = np.float64(0.0)
    pos_sum = np.float64(0.0)
    for r in br.results:
        out = r["res"]
        loss_sum += np.float32(out[:, :MT].sum(dtype=np.float32))
        pos_sum += np.float32(out[:, MT:].sum(dtype=np.float32))
    loss = np.float32(loss_sum / B)
    pos = np.float32(pos_sum)
    return (loss, pos)



# revision 9
# speedup vs baseline: 1.2315x; 1.2315x over previous
"""SimCLR contrastive-loss kernel for 8 Trainium2 NeuronCores.

Full inputs in, full outputs out.  Internally: shard rows across the 8
cores; each core normalizes+transposes its proj_2 shard on the PE (scale
1000/||y|| folded into a diag matmul), AllGathers normalized z2^T in
fp16, computes its 1024x8192 row-block of the 1000x-scaled similarity
matrix with fp16 matmuls (fp32 PSUM accumulate), does a streaming
logsumexp (per-1024-group negated max on DVE feeds the ACT exp bias
directly, exp+row-sum on ACT, exact group-shift fixup at the end), and
writes 1024 per-row losses + 1024 scaled positives.  Host sums partials.
"""

import os
import numpy as np

DEBUG_NO_CC = bool(os.environ.get("K_NO_CC"))

B = 8192          # batch
D = 256           # feature dim
NCORES = 8
R = B // NCORES   # rows per core = 1024
P = 128           # partitions
MT = R // P       # M-tiles per core = 8
GROUP = 1024      # columns per logsumexp group
NG = B // GROUP   # groups per row = 8
TEMP_INV = 1000.0
LN_TEMP_INV = 6.907755278982137  # ln(1000)

_CACHE = {}


def _build_nc():
    import concourse.bacc as bacc
    import concourse.mybir as mybir
    from concourse import tile, masks

    f32 = mybir.dt.float32
    f16 = mybir.dt.float16
    AOT = mybir.AluOpType
    ACT = mybir.ActivationFunctionType
    AXL = mybir.AxisListType

    nc = bacc.Bacc("TRN2", target_bir_lowering=False, debug=False,
                   num_devices=NCORES)

    p1 = nc.dram_tensor("p1", [R, D], f32, kind="ExternalInput")
    p2s = nc.dram_tensor("p2s", [R, D], f32, kind="ExternalInput")
    res = nc.dram_tensor("res", [P, 2 * MT], f32, kind="ExternalOutput")
    ag_in = nc.dram_tensor("ag_in", [D, R], f16, kind="Internal")
    ag_out = nc.dram_tensor("ag_out", [NCORES * D, R], f16, kind="Internal",
                            addr_space="Shared")
    rg = [list(range(NCORES))]

    with tile.TileContext(nc) as tc:
        with (
            tc.tile_pool(name="big", bufs=1) as big,
            tc.tile_pool(name="scr", bufs=2) as scr,
            tc.tile_pool(name="dscr", bufs=4) as dscr,
        ):
            # persistent SBUF tensors
            z2T0 = big.tile([P, B], f16, tag="z2T0")  # z2^T dims 0..127 (x1000)
            z2T1 = big.tile([P, B], f16, tag="z2T1")  # z2^T dims 128..255
            xT0 = big.tile([P, R], f16, tag="xT0")    # x^T dims 0..127 (unit)
            xT1 = big.tile([P, R], f16, tag="xT1")
            xs = big.tile([P, MT * D], f32, tag="xs")  # p1 natural tiles
            ys = big.tile([P, MT * D], f32, tag="ys")  # p2 shard natural tiles
            zsh0 = big.tile([P, R], f16, tag="zsh0")  # staged z2T shard
            zsh1 = big.tile([P, R], f16, tag="zsh1")
            ident = big.tile([P, P], f32, tag="ident")
            n2x = big.tile([P, MT], f32, tag="n2x")
            n2y = big.tile([P, MT], f32, tag="n2y")
            rix = big.tile([P, MT], f32, tag="rix")
            riy = big.tile([P, MT], f32, tag="riy")
            tln = big.tile([P, MT], f32, tag="tln")
            praw = big.tile([P, MT], f32, tag="praw")
            posv = big.tile([P, MT], f32, tag="posv")
            ngb = big.tile([P, MT * NG], f32, tag="ngb")   # -(1000*group max)
            ssum = big.tile([P, MT * NG], f32, tag="ssum")
            t4 = big.tile([P, MT * NG], f32, tag="t4")
            st4 = big.tile([P, MT * NG], f32, tag="st4")
            mn = big.tile([P, MT], f32, tag="mn")          # min_g ngb = -rowmax'
            lnk = big.tile([P, 1], f32, tag="lnk")         # ln(1000) const
            stot = big.tile([P, MT], f32, tag="stot")
            lnst = big.tile([P, MT], f32, tag="lnst")
            wtmp = big.tile([P, MT], f32, tag="wtmp")
            outt = big.tile([P, 2 * MT], f32, tag="outt")

            masks.make_identity(nc, ident[:])
            nc.gpsimd.memset(lnk[:], LN_TEMP_INV)

            # ---------------- phase A: p2 shard -> normalized z2T shard (fp16)
            with tc.tile_pool(name="ppsum", bufs=2, space="PSUM") as ppsum:
                for t in range(MT):
                    nc.sync.dma_start(ys[:, t * D:(t + 1) * D],
                                      p2s[t * P:(t + 1) * P, :])
                for m in range(MT):
                    nc.sync.dma_start(xs[:, m * D:(m + 1) * D],
                                      p1[m * P:(m + 1) * P, :])
                # row norms^2 via Square activation with accumulate
                for t in range(MT):
                    sq = scr.tile([P, D], f32, tag="sq")
                    nc.scalar.activation(sq[:], ys[:, t * D:(t + 1) * D],
                                         ACT.Square,
                                         accum_out=n2y[:, t:t + 1])
                # 1000/sqrt(s) = exp(-0.5*ln(s) + ln(1000))
                nc.scalar.activation(tln[:], n2y[:], ACT.Ln)
                nc.scalar.activation(riy[:], tln[:], ACT.Exp, scale=-0.5,
                                     bias=lnk[:, 0:1])
                pt0 = ppsum.tile([P, R], f32, tag="pt0")
                pt1 = ppsum.tile([P, R], f32, tag="pt1")
                for t in range(MT):
                    dg = dscr.tile([P, P], f32, tag="dg")
                    nc.gpsimd.tensor_scalar_mul(dg[:], ident[:],
                                                riy[:, t:t + 1])
                    nc.tensor.matmul(pt0[:, t * P:(t + 1) * P],
                                     ys[:, t * D:t * D + P], dg[:])
                    nc.tensor.matmul(pt1[:, t * P:(t + 1) * P],
                                     ys[:, t * D + P:(t + 1) * D], dg[:])
                nc.vector.tensor_copy(zsh0[:], pt0[:])
                nc.vector.tensor_copy(zsh1[:], pt1[:])

                # ship z2T shard to DRAM, AllGather (runs on TOPSP/SDMA)
                nc.sync.dma_start(ag_in[0:P, :], zsh0[:])
                nc.sync.dma_start(ag_in[P:D, :], zsh1[:])
                if not DEBUG_NO_CC:
                    nc.gpsimd.collective_compute(
                        "AllGather", AOT.bypass, replica_groups=rg,
                        ins=[ag_in.ap()], outs=[ag_out.ap()])

                # ---------------- phase B: p1 shard -> normalized x^T (fp16)
                # (overlaps the AllGather)
                for m in range(MT):
                    sq = scr.tile([P, D], f32, tag="sq")
                    nc.scalar.activation(sq[:], xs[:, m * D:(m + 1) * D],
                                         ACT.Square,
                                         accum_out=n2x[:, m:m + 1])
                nc.scalar.activation(tln[:], n2x[:], ACT.Ln)
                nc.scalar.activation(rix[:], tln[:], ACT.Exp, scale=-0.5)
                pt0 = ppsum.tile([P, R], f32, tag="pt0")
                pt1 = ppsum.tile([P, R], f32, tag="pt1")
                for m in range(MT):
                    dg = dscr.tile([P, P], f32, tag="dg")
                    nc.gpsimd.tensor_scalar_mul(dg[:], ident[:],
                                                rix[:, m:m + 1])
                    nc.tensor.matmul(pt0[:, m * P:(m + 1) * P],
                                     xs[:, m * D:m * D + P], dg[:])
                    nc.tensor.matmul(pt1[:, m * P:(m + 1) * P],
                                     xs[:, m * D + P:(m + 1) * D], dg[:])
                nc.vector.tensor_copy(xT0[:], pt0[:])
                nc.vector.tensor_copy(xT1[:], pt1[:])

            # positives (exact fp32, from raw shards); posv = 1000*pos
            for m in range(MT):
                sq = scr.tile([P, D], f32, tag="sq")
                nc.gpsimd.tensor_mul(sq[:], xs[:, m * D:(m + 1) * D],
                                     ys[:, m * D:(m + 1) * D])
                nc.vector.reduce_sum(out=praw[:, m:m + 1], in_=sq[:],
                                     axis=AXL.X)
            nc.vector.tensor_mul(posv[:], rix[:], riy[:])
            nc.vector.tensor_mul(posv[:], praw[:], posv[:])

            # pull gathered z2T into SBUF (chunked)
            if DEBUG_NO_CC:
                for c in range(NCORES):
                    nc.sync.dma_start(z2T0[:, c * R:(c + 1) * R],
                                      ag_in[0:P, :])
                    nc.sync.dma_start(z2T1[:, c * R:(c + 1) * R],
                                      ag_in[P:D, :])
            else:
                for c in range(NCORES):
                    nc.sync.dma_start(z2T0[:, c * R:(c + 1) * R],
                                      ag_out[c * D:c * D + P, :])
                    nc.sync.dma_start(z2T1[:, c * R:(c + 1) * R],
                                      ag_out[c * D + P:(c + 1) * D, :])

            # ---------------- main loop: row-block scaled sim + logsumexp
            xTk = (xT0, xT1)
            zTk = (z2T0, z2T1)
            with (tc.tile_pool(name="mpsum", bufs=4, space="PSUM") as mpsum,
                  tc.tile_pool(name="escr", bufs=3) as escr):
                for m in range(MT):
                    for g in range(NG):
                        col = m * NG + g
                        pg = mpsum.tile([P, GROUP], f32, tag="pg")
                        for k in range(2):
                            for n in range(GROUP // 512):
                                nc.tensor.matmul(
                                    pg[:, n * 512:(n + 1) * 512],
                                    xTk[k][:, m * P:(m + 1) * P],
                                    zTk[k][:, g * GROUP + n * 512:
                                            g * GROUP + (n + 1) * 512],
                                    start=(k == 0), stop=(k == 1))
                        nc.vector.reduce_max(out=ngb[:, col:col + 1],
                                             in_=pg[:],
                                             axis=AXL.X, negate=True)
                        eo = escr.tile([P, GROUP], f16, tag="eo")
                        nc.scalar.activation(eo[:], pg[:], ACT.Exp,
                                             bias=ngb[:, col:col + 1],
                                             accum_out=ssum[:, col:col + 1])

            # ---------------- fixup: combine the NG group sums exactly
            for m in range(MT):
                c0, c1 = m * NG, (m + 1) * NG
                nc.vector.tensor_reduce(out=mn[:, m:m + 1],
                                        in_=ngb[:, c0:c1],
                                        axis=AXL.X, op=AOT.min)
                # exp(gmax' - rowmax') = exp(-ngb + mn)
                nc.scalar.activation(t4[:, c0:c1], ngb[:, c0:c1],
                                     ACT.Exp, scale=-1.0,
                                     bias=mn[:, m:m + 1])
            nc.vector.tensor_mul(st4[:], t4[:], ssum[:])
            for m in range(MT):
                c0, c1 = m * NG, (m + 1) * NG
                nc.vector.reduce_sum(out=stot[:, m:m + 1],
                                     in_=st4[:, c0:c1],
                                     axis=AXL.X)
            nc.scalar.activation(lnst[:], stot[:], ACT.Ln)

            # all_losses = ln(stot) + rowmax' - posv'  (rowmax' = -mn)
            nc.vector.tensor_sub(wtmp[:], lnst[:], mn[:])
            nc.vector.tensor_sub(outt[:, 0:MT], wtmp[:], posv[:])
            nc.vector.tensor_copy(outt[:, MT:2 * MT], posv[:])
            nc.sync.dma_start(res[:, :], outt[:])

    nc.compile()
    return nc


def _get_nc():
    if "nc" not in _CACHE:
        _CACHE["nc"] = _build_nc()
    return _CACHE["nc"]


def run_cores(proj_1, proj_2, **spmd_kwargs):
    """Run the SPMD kernel; returns BassKernelResults."""
    from concourse.bass_utils import run_bass_kernel_spmd

    p1 = np.ascontiguousarray(np.asarray(proj_1, dtype=np.float32))
    p2 = np.ascontiguousarray(np.asarray(proj_2, dtype=np.float32))
    assert p1.shape == (B, D) and p2.shape == (B, D)
    in_maps = [
        {"p1": p1[c * R:(c + 1) * R], "p2s": p2[c * R:(c + 1) * R]}
        for c in range(NCORES)
    ]
    nc = _get_nc()
    br = run_bass_kernel_spmd(nc, in_maps, core_ids=list(range(NCORES)),
                              **spmd_kwargs)
    return br


def kernel(proj_1, proj_2):
    br = run_cores(proj_1, proj_2)
    loss_sum = np.float64(0.0)
    pos_sum = np.float64(0.0)
    for r in br.results:
        out = r["res"]
        loss_sum += np.float64(out[:, :MT].astype(np.float64).sum())
        pos_sum += np.float64(out[:, MT:].astype(np.float64).sum())
    loss = np.float32(loss_sum / B)
    pos = np.float32(pos_sum / TEMP_INV)
    return (loss, pos)
